# revision 3
# baseline (speedup 1.0000x reference)
"""Bipartite NAND/NOR graph layer on 8 Trainium2 NeuronCores.

Problem: out[i] = ~(x[a_i] & x[b_i]) if not nor_mask[i] else ~(x[a_i] | x[b_i])
with x: [32768, 2048] int32, (a, b): [32768, 2] indices, nor_mask: [32768] bool.

Strategy (word-sharded, zero cross-core communication):
- Each core owns a 256-word column slice of x; indices/mask are replicated.
- Host passes the COMPLEMENTED table cx = ~x.  By De Morgan:
      NAND: ~(a & b) = ~a | ~b = cx_a | cx_b
      NOR:  ~(a | b) = ~a & ~b = cx_a & cx_b
  so each output row is a single OR/AND of two gathered complemented rows.
- Output rows are processed grouped by mask value (all-NAND chunks first,
  then all-NOR chunks) so each 2048-row chunk needs exactly one vector op.
  The device stores chunks contiguously in permuted order; the host
  scatters rows back to their original positions while assembling the
  full output.
- Gathers use the gpsimd dma_gather ucode op (int16 indices wrapped in 16
  partitions): row j of a chunk lands at SBUF [j % 128, j // 128, :].
"""
import sys
sys.path.insert(0, "/opt/trn_rl_repo")

import numpy as np
from contextlib import ExitStack

import concourse.bass as bass
import concourse.tile as tile
from concourse import bacc, mybir
from concourse.bass_utils import run_bass_kernel_spmd

N_ROWS = 32768          # input rows == output rows
W_FULL = 2048           # int32 words per row
N_CORES = 8
WS = W_FULL // N_CORES  # 256 words per core slice (1 KiB)
P = 128
CHUNK = 1024            # rows per dma_gather call (ucode limit: 2048 crashes)
B = CHUNK // P          # free-dim blocks per chunk tile
IDX_COLS = CHUNK // 16  # int16 index columns per chunk
GATHER_QUEUES = (0, 1)
NUM_SWDGE_QUEUES = 2


def _wrap_idxs(idx_chunk):
    """[CHUNK] int (-1 padded suffix) -> [128, CHUNK//16] int16 wrapped in 16
    partitions, replicated across the 8 gpsimd core windows."""
    blk = idx_chunk.reshape(IDX_COLS, 16).T.astype(np.int16)  # [16, S]
    return np.tile(blk, (8, 1))


def _build(chunk_ops, chunk_valid, n_chunks, repeats=1):
    """chunk_ops[c] in {'or','and'}; chunk_valid[c] = non-padded row count.

    repeats>1 wraps the whole chunk loop in a hardware loop — used only for
    differential wall-clock timing of the on-device portion."""
    L = n_chunks * CHUNK
    nc = bacc.Bacc("TRN2", target_bir_lowering=False, debug=False,
                   num_devices=N_CORES, num_swdge_queues=NUM_SWDGE_QUEUES)
    x = nc.dram_tensor("x", [N_ROWS, WS], mybir.dt.int32,
                       kind="ExternalInput").ap()
    ia = nc.dram_tensor("ia", [P, n_chunks * IDX_COLS], mybir.dt.int16,
                        kind="ExternalInput").ap()
    ib = nc.dram_tensor("ib", [P, n_chunks * IDX_COLS], mybir.dt.int16,
                        kind="ExternalInput").ap()
    out = nc.dram_tensor("out", [L, WS], mybir.dt.int32,
                         kind="ExternalOutput").ap()
    with ExitStack() as ctx:
        tc = ctx.enter_context(tile.TileContext(nc))
        idxp = ctx.enter_context(tc.tile_pool(name="idx", bufs=1))
        datap = ctx.enter_context(tc.tile_pool(name="data", bufs=3))
        ta_i = idxp.tile([P, n_chunks * IDX_COLS], mybir.dt.int16)
        tb_i = idxp.tile([P, n_chunks * IDX_COLS], mybir.dt.int16)
        nc.sync.dma_start(ta_i[:], ia)
        nc.sync.dma_start(tb_i[:], ib)
        loop = tc.For_i(0, repeats, 1) if repeats > 1 else None
        if loop is not None:
            loop.__enter__()
        for c in range(n_chunks):
            isl = slice(c * IDX_COLS, (c + 1) * IDX_COLS)
            ta = datap.tile([P, B, WS], mybir.dt.int32, tag="ta")
            nc.gpsimd.dma_gather(
                out_ap=ta[:, :, :], in_ap=x, idxs_ap=ta_i[:, isl],
                num_idxs=CHUNK, num_idxs_reg=int(chunk_valid[c]),
                elem_size=WS, queue_num=GATHER_QUEUES[0])
            tb = datap.tile([P, B, WS], mybir.dt.int32, tag="tb")
            nc.gpsimd.dma_gather(
                out_ap=tb[:, :, :], in_ap=x, idxs_ap=tb_i[:, isl],
                num_idxs=CHUNK, num_idxs_reg=int(chunk_valid[c]),
                elem_size=WS, queue_num=GATHER_QUEUES[1])
            r = datap.tile([P, B, WS], mybir.dt.int32, tag="r")
            op = (mybir.AluOpType.bitwise_or if chunk_ops[c] == 'or'
                  else mybir.AluOpType.bitwise_and)
            nc.vector.tensor_tensor(out=r[:, :, :], in0=ta[:, :, :],
                                    in1=tb[:, :, :], op=op)
            nc.sync.dma_start(
                out[c * CHUNK:(c + 1) * CHUNK, :].rearrange(
                    '(b p) w -> p b w', b=B, p=P),
                r[:, :, :])
        if loop is not None:
            loop.__exit__(None, None, None)
    nc.finalize()
    return nc


def _prepare(output_node_input_indices, nor_mask):
    """Group rows by mask, pad each group to CHUNK multiples.

    Returns (ia, ib, chunk_ops, chunk_valid, row_order) where row_order[k] is
    the original output row stored at device position k (-1 for padding)."""
    idx = np.asarray(output_node_input_indices)
    mask = np.asarray(nor_mask).astype(bool)
    # AND/OR are commutative: put the smaller index in operand a, then order
    # rows by it.  The a-gather then reads HBM nearly sequentially (each
    # chunk touches a narrow ascending row window) instead of randomly.
    lo = np.minimum(idx[:, 0], idx[:, 1]).astype(np.int64)
    nand_rows = np.flatnonzero(~mask)
    nor_rows = np.flatnonzero(mask)
    nand_rows = nand_rows[np.argsort(lo[nand_rows], kind="stable")]
    nor_rows = nor_rows[np.argsort(lo[nor_rows], kind="stable")]

    streams, chunk_ops = [], []
    for rows, op in ((nand_rows, 'or'), (nor_rows, 'and')):
        n_chunks = max(1, -(-len(rows) // CHUNK))
        padded = np.full(n_chunks * CHUNK, -1, dtype=np.int64)
        padded[:len(rows)] = rows
        streams.append(padded)
        chunk_ops.extend([op] * n_chunks)
    row_order = np.concatenate(streams)
    n_chunks = len(chunk_ops)

    safe = np.clip(row_order, 0, None)
    a_sm = np.minimum(idx[safe, 0], idx[safe, 1])
    b_lg = np.maximum(idx[safe, 0], idx[safe, 1])
    a_full = np.where(row_order >= 0, a_sm, -1)
    b_full = np.where(row_order >= 0, b_lg, -1)
    chunk_valid = [(row_order[c * CHUNK:(c + 1) * CHUNK] >= 0).sum()
                   for c in range(n_chunks)]
    ia = np.concatenate(
        [_wrap_idxs(a_full[c * CHUNK:(c + 1) * CHUNK]) for c in range(n_chunks)],
        axis=1)
    ib = np.concatenate(
        [_wrap_idxs(b_full[c * CHUNK:(c + 1) * CHUNK]) for c in range(n_chunks)],
        axis=1)
    return ia, ib, chunk_ops, chunk_valid, row_order


def kernel(input_bitarrays, output_node_input_indices, nor_mask):
    x = np.asarray(input_bitarrays)
    assert x.shape == (N_ROWS, W_FULL) and x.dtype == np.int32
    ia, ib, chunk_ops, chunk_valid, row_order = _prepare(
        output_node_input_indices, nor_mask)
    n_chunks = len(chunk_ops)

    cx = ~x  # complemented table; gathers+single AND/OR give NAND/NOR directly
    in_maps = []
    for c in range(N_CORES):
        in_maps.append({
            "x": np.ascontiguousarray(cx[:, c * WS:(c + 1) * WS]),
            "ia": ia,
            "ib": ib,
        })

    nc = _build(chunk_ops, chunk_valid, n_chunks)
    res = run_bass_kernel_spmd(nc, in_maps, core_ids=list(range(N_CORES)))

    valid = row_order >= 0
    rows = row_order[valid]
    result = np.empty((N_ROWS, W_FULL), dtype=np.int32)
    for c in range(N_CORES):
        result[rows, c * WS:(c + 1) * WS] = res.results[c]["out"][valid]
    return result



# revision 6
# speedup vs baseline: 1.1257x; 1.1257x over previous
"""Bipartite NAND/NOR graph layer on 8 Trainium2 NeuronCores.

Problem: out[i] = ~(x[a_i] & x[b_i]) if not nor_mask[i] else ~(x[a_i] | x[b_i])
with x: [32768, 2048] int32, (a, b): [32768, 2] indices, nor_mask: [32768] bool.

Strategy (word-sharded, zero cross-core communication):
- Each core owns a 256-word column slice of x; indices/mask are replicated.
- Host passes the COMPLEMENTED table cx = ~x.  By De Morgan:
      NAND: ~(a & b) = ~a | ~b = cx_a | cx_b
      NOR:  ~(a | b) = ~a & ~b = cx_a & cx_b
  so each output row is a single OR/AND of two gathered complemented rows.
- Output rows are processed grouped by mask value (all-NAND chunks first,
  then all-NOR chunks) so each 2048-row chunk needs exactly one vector op.
  The device stores chunks contiguously in permuted order; the host
  scatters rows back to their original positions while assembling the
  full output.
- Gathers use the gpsimd dma_gather ucode op (int16 indices wrapped in 16
  partitions): row j of a chunk lands at SBUF [j % 128, j // 128, :].
"""
import sys
sys.path.insert(0, "/opt/trn_rl_repo")

import numpy as np
from contextlib import ExitStack

import concourse.bass as bass
import concourse.tile as tile
from concourse import bacc, mybir
from concourse.bass_utils import run_bass_kernel_spmd

N_ROWS = 32768          # input rows == output rows
W_FULL = 2048           # int32 words per row
N_CORES = 8
WS = W_FULL // N_CORES  # 256 words per core slice (1 KiB)
P = 128
CHUNK = 1024            # rows per dma_gather call (ucode limit: 2048 crashes)
B = CHUNK // P          # free-dim blocks per chunk tile
IDX_COLS = CHUNK // 16  # int16 index columns per chunk
# 4 SWDGE queues + 7-deep tile pool measured 324 us/pass vs 389 us for the
# 2-queue/3-buf config (differential wall-clock, bench.py).
GATHER_QUEUES = (0, 1, 2, 3)
NUM_SWDGE_QUEUES = 4
DATA_BUFS = 7


def _wrap_idxs(idx_chunk):
    """[CHUNK] int (-1 padded suffix) -> [128, CHUNK//16] int16 wrapped in 16
    partitions, replicated across the 8 gpsimd core windows."""
    blk = idx_chunk.reshape(IDX_COLS, 16).T.astype(np.int16)  # [16, S]
    return np.tile(blk, (8, 1))


def _build(chunk_ops, chunk_valid, n_chunks, repeats=1):
    """chunk_ops[c] in {'or','and'}; chunk_valid[c] = non-padded row count.

    repeats>1 wraps the whole chunk loop in a hardware loop — used only for
    differential wall-clock timing of the on-device portion."""
    L = n_chunks * CHUNK
    nc = bacc.Bacc("TRN2", target_bir_lowering=False, debug=False,
                   num_devices=N_CORES, num_swdge_queues=NUM_SWDGE_QUEUES)
    x = nc.dram_tensor("x", [N_ROWS, WS], mybir.dt.int32,
                       kind="ExternalInput").ap()
    ia = nc.dram_tensor("ia", [P, n_chunks * IDX_COLS], mybir.dt.int16,
                        kind="ExternalInput").ap()
    ib = nc.dram_tensor("ib", [P, n_chunks * IDX_COLS], mybir.dt.int16,
                        kind="ExternalInput").ap()
    out = nc.dram_tensor("out", [L, WS], mybir.dt.int32,
                         kind="ExternalOutput").ap()
    with ExitStack() as ctx:
        tc = ctx.enter_context(tile.TileContext(nc))
        idxp = ctx.enter_context(tc.tile_pool(name="idx", bufs=1))
        datap = ctx.enter_context(tc.tile_pool(name="data", bufs=DATA_BUFS))
        ta_i = idxp.tile([P, n_chunks * IDX_COLS], mybir.dt.int16)
        tb_i = idxp.tile([P, n_chunks * IDX_COLS], mybir.dt.int16)
        nc.sync.dma_start(ta_i[:], ia)
        nc.sync.dma_start(tb_i[:], ib)
        loop = tc.For_i(0, repeats, 1) if repeats > 1 else None
        if loop is not None:
            loop.__enter__()
        nq = len(GATHER_QUEUES)
        for c in range(n_chunks):
            isl = slice(c * IDX_COLS, (c + 1) * IDX_COLS)
            ta = datap.tile([P, B, WS], mybir.dt.int32, tag="ta")
            nc.gpsimd.dma_gather(
                out_ap=ta[:, :, :], in_ap=x, idxs_ap=ta_i[:, isl],
                num_idxs=CHUNK, num_idxs_reg=int(chunk_valid[c]),
                elem_size=WS, queue_num=GATHER_QUEUES[(2 * c) % nq])
            tb = datap.tile([P, B, WS], mybir.dt.int32, tag="tb")
            nc.gpsimd.dma_gather(
                out_ap=tb[:, :, :], in_ap=x, idxs_ap=tb_i[:, isl],
                num_idxs=CHUNK, num_idxs_reg=int(chunk_valid[c]),
                elem_size=WS, queue_num=GATHER_QUEUES[(2 * c + 1) % nq])
            r = datap.tile([P, B, WS], mybir.dt.int32, tag="r")
            op = (mybir.AluOpType.bitwise_or if chunk_ops[c] == 'or'
                  else mybir.AluOpType.bitwise_and)
            nc.vector.tensor_tensor(out=r[:, :, :], in0=ta[:, :, :],
                                    in1=tb[:, :, :], op=op)
            nc.sync.dma_start(
                out[c * CHUNK:(c + 1) * CHUNK, :].rearrange(
                    '(b p) w -> p b w', b=B, p=P),
                r[:, :, :])
        if loop is not None:
            loop.__exit__(None, None, None)
    nc.finalize()
    return nc


def _prepare(output_node_input_indices, nor_mask):
    """Group rows by mask, pad each group to CHUNK multiples.

    Returns (ia, ib, chunk_ops, chunk_valid, row_order) where row_order[k] is
    the original output row stored at device position k (-1 for padding)."""
    idx = np.asarray(output_node_input_indices)
    mask = np.asarray(nor_mask).astype(bool)
    # AND/OR are commutative: put the smaller index in operand a, then order
    # rows by it.  The a-gather then reads HBM nearly sequentially (each
    # chunk touches a narrow ascending row window) instead of randomly.
    lo = np.minimum(idx[:, 0], idx[:, 1]).astype(np.int64)
    nand_rows = np.flatnonzero(~mask)
    nor_rows = np.flatnonzero(mask)
    nand_rows = nand_rows[np.argsort(lo[nand_rows], kind="stable")]
    nor_rows = nor_rows[np.argsort(lo[nor_rows], kind="stable")]

    streams, chunk_ops = [], []
    for rows, op in ((nand_rows, 'or'), (nor_rows, 'and')):
        n_chunks = max(1, -(-len(rows) // CHUNK))
        padded = np.full(n_chunks * CHUNK, -1, dtype=np.int64)
        padded[:len(rows)] = rows
        streams.append(padded)
        chunk_ops.extend([op] * n_chunks)
    row_order = np.concatenate(streams)
    n_chunks = len(chunk_ops)

    safe = np.clip(row_order, 0, None)
    a_sm = np.minimum(idx[safe, 0], idx[safe, 1])
    b_lg = np.maximum(idx[safe, 0], idx[safe, 1])
    a_full = np.where(row_order >= 0, a_sm, -1)
    b_full = np.where(row_order >= 0, b_lg, -1)
    chunk_valid = [(row_order[c * CHUNK:(c + 1) * CHUNK] >= 0).sum()
                   for c in range(n_chunks)]
    ia = np.concatenate(
        [_wrap_idxs(a_full[c * CHUNK:(c + 1) * CHUNK]) for c in range(n_chunks)],
        axis=1)
    ib = np.concatenate(
        [_wrap_idxs(b_full[c * CHUNK:(c + 1) * CHUNK]) for c in range(n_chunks)],
        axis=1)
    return ia, ib, chunk_ops, chunk_valid, row_order


def kernel(input_bitarrays, output_node_input_indices, nor_mask):
    x = np.asarray(input_bitarrays)
    assert x.shape == (N_ROWS, W_FULL) and x.dtype == np.int32
    ia, ib, chunk_ops, chunk_valid, row_order = _prepare(
        output_node_input_indices, nor_mask)
    n_chunks = len(chunk_ops)

    cx = ~x  # complemented table; gathers+single AND/OR give NAND/NOR directly
    in_maps = []
    for c in range(N_CORES):
        in_maps.append({
            "x": np.ascontiguousarray(cx[:, c * WS:(c + 1) * WS]),
            "ia": ia,
            "ib": ib,
        })

    nc = _build(chunk_ops, chunk_valid, n_chunks)
    res = run_bass_kernel_spmd(nc, in_maps, core_ids=list(range(N_CORES)))

    valid = row_order >= 0
    rows = row_order[valid]
    result = np.empty((N_ROWS, W_FULL), dtype=np.int32)
    for c in range(N_CORES):
        result[rows, c * WS:(c + 1) * WS] = res.results[c]["out"][valid]
    return result

